# revision 28
# baseline (speedup 1.0000x reference)
"""Galerkin-attention encoder block on 8 TRN2 NeuronCores.

Sharding: tokens (N=8192 -> 1024/core). The only cross-core dependency is
the Galerkin contraction scores[b,h] = sum_n k[n] (x) v[n] / N, reduced with
four per-batch 512KB AllReduces that overlap local compute.

All device compute runs in "transposed space" (features on partitions,
tokens on the free axis) against host-side pre-transposed x^T, so the
kernel needs no on-device transposes anywhere.

Precision/speed scheme: every large matmul (QKV projections, FFN1, FFN2)
runs in fp8e4m3 with DoubleRow perf mode (2 contraction chunks per
instruction, ~1.9x the bf16 PE throughput). Weights are pre-scaled by 32
into fp8's normal range; the 1/32 descale folds into the activation-scale
of the op that drains each PSUM accumulation. K/V feed LayerNorm, which is
scale-invariant, so their descale is free. The fp8 noise of the FFN is
tamed by routing the linear bulk of SiLU through a precise bf16 bypass:
  silu(z) = z/2 + g(z),  g(z) = (z/2)*tanh(z/2)
  ffn_out = x1 @ (0.5*W1@W2) [bf16, D x D]  +  g(z) @ W2 [fp8]
W12 = 0.5*W1@W2 is computed host-side; the bypass matmuls accumulate into
the same PSUM group as the fp8 FFN2 matmuls. Only g (|g| ~ 0.44|h|) and
its upstream fp8 z-noise (damped by g' ~ 0.3 instead of silu' ~ 0.8) pass
through fp8, cutting the FFN quantization error ~2.2x. The residual stream
x, x1 stays bf16 end to end.

Phase B is fully fused per 512-token super-tile: q -> attn -> x1 -> FFN1
-> g -> FFN2(+bypass) -> out, with g and x1 living entirely in SBUF.
"""

import numpy as np
import ml_dtypes

B, N, D = 4, 8192, 1024
H, DK = 8, 128
FF = 4096
EPS = 1e-5
N_CORES = 8
NT = N // N_CORES          # tokens per core
KC = D // 128              # feature chunks of 128
FC = FF // 128
SUP = 512                  # tokens per super-tile in phase B
NSUP = NT // SUP
SUB = 128                  # tokens per sub-tile in phase A
NSUB = SUP // SUB
SW = 32.0                  # fp8 weight pre-scale

_GRAPH_CACHE = {}


def _build(flags):
    import concourse.bass as bass
    import concourse.tile as tile
    from concourse import bacc, mybir
    from contextlib import ExitStack

    has_bk, has_bv, has_b1, has_b2, has_affine = flags
    f32 = mybir.dt.float32
    bf16 = mybir.dt.bfloat16
    f8 = mybir.dt.float8e4
    DR = mybir.MatmulPerfMode.DoubleRow

    nc = bacc.Bacc("TRN2", target_bir_lowering=False, debug=False,
                   num_devices=N_CORES)

    # All tensors arrive pre-tiled in their exact SBUF layouts so every DMA
    # is a contiguous-per-partition copy (one descriptor line per partition
    # instead of thousands — sequencer descriptor issue was costing ~30us).
    xTb_d = nc.dram_tensor("xTb", [B, NSUP, 128, KC, SUP], f8, kind="ExternalInput")
    xTbBf_d = nc.dram_tensor("xTbBf", [B, NSUP, 128, KC, SUP], bf16, kind="ExternalInput")
    delta_d = nc.dram_tensor("delta", [128, NT // 128], f32, kind="ExternalInput")
    wq_d = nc.dram_tensor("Wq", [128, KC, D], f8, kind="ExternalInput")
    wk_d = nc.dram_tensor("Wk", [128, KC, D], f8, kind="ExternalInput")
    wv_d = nc.dram_tensor("Wv", [128, KC, D], f8, kind="ExternalInput")
    w1_d = nc.dram_tensor("W1", [128, KC, FF], f8, kind="ExternalInput")
    w2_d = nc.dram_tensor("W2", [128, FC, D], f8, kind="ExternalInput")
    w12_d = nc.dram_tensor("W12", [128, KC, D], bf16, kind="ExternalInput")
    bq_d = nc.dram_tensor("bq", [128, KC], f32, kind="ExternalInput")
    b1_d = nc.dram_tensor("b1", [128, FC], f32, kind="ExternalInput")
    b1h_d = nc.dram_tensor("b1h", [128, FC], f32, kind="ExternalInput") if has_b1 else None
    bk_d = nc.dram_tensor("bk", [D], f32, kind="ExternalInput") if has_bk else None
    bv_d = nc.dram_tensor("bv", [D], f32, kind="ExternalInput") if has_bv else None
    b2_d = nc.dram_tensor("b2", [128, KC], f32, kind="ExternalInput") if has_b2 else None
    gamma_d = nc.dram_tensor("gamma", [D], f32, kind="ExternalInput") if has_affine else None
    beta_d = nc.dram_tensor("beta", [D], f32, kind="ExternalInput") if has_affine else None
    out_d = nc.dram_tensor("outT", [B, NSUP, 128, KC, SUP], bf16, kind="ExternalOutput")

    sub_ = mybir.AluOpType.subtract
    mult = mybir.AluOpType.mult
    add_ = mybir.AluOpType.add
    ACT = mybir.ActivationFunctionType

    with tile.TileContext(nc) as tc, ExitStack() as ctx:
        singles = ctx.enter_context(tc.tile_pool(name="singles", bufs=1))
        dram = ctx.enter_context(tc.tile_pool(name="dram", bufs=1, space="DRAM"))

        eps_t = singles.tile([128, 1], f32)
        nc.vector.memset(eps_t, EPS)
        delta_sb = singles.tile([128, NT // 128], f32)
        nc.sync.dma_start(out=delta_sb[:], in_=delta_d.ap())
        scores_bf = singles.tile([128, B, H, DK], bf16)

        cc_in = dram.tile([B, 128, H * DK], f32)
        cc_out = [dram.tile([128, H * DK], f32, addr_space="Shared",
                            name=f"cc_out{b}") for b in range(B)]

        # Weights that live through phase B only; their DMAs ride the scalar
        # queue early so they are resident long before B starts without
        # delaying Wk/Wv/x on sync — those gate the very first matmuls.
        w_b2a_cm = tc.tile_pool(name="w_b2a", bufs=1)
        w_b2a = w_b2a_cm.__enter__()
        w2_sb = w_b2a.tile([128, FC, D], f8)
        nc.scalar.dma_start(
            out=w2_sb[:], in_=w2_d.ap())
        w12_sb = w_b2a.tile([128, KC, D], bf16)
        nc.scalar.dma_start(
            out=w12_sb[:], in_=w12_d.ap())

        w_ab1_cm = tc.tile_pool(name="w_ab1", bufs=1)
        w_ab1 = w_ab1_cm.__enter__()
        wq_sb = w_ab1.tile([128, KC, D], f8)
        nc.scalar.dma_start(out=wq_sb[:], in_=wq_d.ap())
        w1_sb = w_ab1.tile([128, KC, FF], f8)
        nc.scalar.dma_start(out=w1_sb[:], in_=w1_d.ap())
        bq_sb = w_ab1.tile([128, KC], f32)
        nc.scalar.dma_start(out=bq_sb[:], in_=bq_d.ap())
        b1_sb = w_ab1.tile([128, FC], f32)
        nc.scalar.dma_start(out=b1_sb[:], in_=b1_d.ap())
        if has_b1:
            b1h_sb = w_ab1.tile([128, FC], f32)
            nc.scalar.dma_start(out=b1h_sb[:], in_=b1h_d.ap())
        if has_b2:
            b2_sb = w_ab1.tile([128, KC], f32)
            nc.scalar.dma_start(out=b2_sb[:], in_=b2_d.ap())
        if has_affine:
            gamma_sb = w_ab1.tile([128, D], f32)
            nc.scalar.dma_start(out=gamma_sb[:], in_=gamma_d.ap().to_broadcast([128, D]))
            beta_sb = w_ab1.tile([128, D], f32)
            nc.scalar.dma_start(out=beta_sb[:], in_=beta_d.ap().to_broadcast([128, D]))
        if has_bk:
            bk_sb = w_ab1.tile([128, D], f32)
            nc.scalar.dma_start(out=bk_sb[:], in_=bk_d.ap().to_broadcast([128, D]))
        if has_bv:
            bv_sb = w_ab1.tile([128, D], f32)
            nc.scalar.dma_start(out=bv_sb[:], in_=bv_d.ap().to_broadcast([128, D]))

        # ---------------- Phase A: k, v, LN, partial scores, AllReduce ----
        with (
            tc.tile_pool(name="wa", bufs=1) as wa,
            tc.tile_pool(name="a_x", bufs=2) as a_x,
            tc.tile_pool(name="a_kvf", bufs=4) as a_kvf,
            tc.tile_pool(name="a_ln", bufs=8) as a_ln,
            tc.tile_pool(name="a_sc", bufs=2) as a_sc,
            tc.tile_pool(name="a_kvps", bufs=6, space="PSUM") as a_kvps,
            tc.tile_pool(name="a_sps", bufs=1, space="PSUM") as a_sps,
        ):
            # First super-tile of x first: it gates the very first matmul.
            xtb0 = a_x.tile([128, KC, SUP], f8, tag="xtb")
            nc.sync.dma_start(out=xtb0[:], in_=xTb_d.ap()[0, 0])
            wk_sb = wa.tile([128, KC, D], f8)
            nc.sync.dma_start(out=wk_sb[:], in_=wk_d.ap())
            wv_sb = wa.tile([128, KC, D], f8)
            nc.gpsimd.dma_start(out=wv_sb[:], in_=wv_d.ap())

            scores_tiles = {}
            pending = None  # (halves, batch, is_last_subtile_of_batch)

            def emit_scores(p):
                # PE executes its queue in order, so a subtile's scores
                # matmuls (gated by the LN chain) are deferred until after
                # the NEXT subtile's projections are queued — the LN latency
                # hides behind ~4us of projection work; the deferral spans
                # batch boundaries.
                halves, pb, last = p
                sp = scores_tiles[pb]
                for oc in range(2):
                    kf, vf = halves[oc]
                    for h in range(4):
                        ph = slice(h * DK, (h + 1) * DK)
                        nc.tensor.matmul(
                            sp[:, oc * 4 + h, :], lhsT=kf[:, ph], rhs=vf[:, ph],
                            start=False, stop=last, skip_group_check=True)
                if last:
                    sc_sb = a_sc.tile([128, H * DK], f32, tag="scsb")
                    nc.vector.tensor_copy(out=sc_sb[:], in_=sp[:, :, :])
                    nc.sync.dma_start(out=cc_in[pb], in_=sc_sb[:])
                    nc.gpsimd.collective_compute(
                        "AllReduce", mybir.AluOpType.add,
                        replica_groups=[list(range(N_CORES))],
                        ins=[cc_in[pb].opt()], outs=[cc_out[pb].opt()])

            for b in range(B):
                for s in range(NSUP):
                    if b == 0 and s == 0:
                        xtb = xtb0
                    else:
                        xtb = a_x.tile([128, KC, SUP], f8, tag="xtb")
                        nc.sync.dma_start(out=xtb[:], in_=xTb_d.ap()[b, s])
                    for sb in range(NSUB):
                        gsub = s * NSUB + sb
                        tsl = bass.ts(sb, SUB)

                        def half_proj(w_sb, bias_sb, oc, tag):
                            # One 512-feature half of a K/V projection: a
                            # single PSUM bank, releasable right after its
                            # 4-head LN half is applied.
                            ps = a_kvps.tile([128, 512], f32, tag="kv")
                            for kc in range(0, KC, 2):
                                nc.tensor.matmul(
                                    ps[:],
                                    lhsT=xtb[:, kc:kc + 2, tsl],
                                    rhs=w_sb[:, kc:kc + 2, oc * 512:(oc + 1) * 512],
                                    start=(kc == 0), stop=(kc == KC - 2),
                                    perf_mode=DR)
                            if bias_sb is not None:
                                nc.vector.tensor_add(ps[:], ps[:],
                                                     bias_sb[:, oc * 512:(oc + 1) * 512])
                            return ps

                        halves = []
                        for oc in range(2):
                            # K and V halves together: one batched sqrt and
                            # reciprocal serve all 8 stats; K's LN apply runs
                            # on Scalar, V's on DVE.
                            k_ps = half_proj(wk_sb, bk_sb if has_bk else None, oc, "k")
                            v_ps = half_proj(wv_sb, bv_sb if has_bv else None, oc, "v")
                            kf = a_kvf.tile([128, 512], bf16, tag="kf")
                            vf = a_kvf.tile([128, 512], bf16, tag="vf")

                            stats = a_ln.tile([128, 8, 6], f32, tag="stats")
                            mv = a_ln.tile([128, 8, 2], f32, tag="mv")
                            for h in range(4):
                                nc.vector.bn_stats(out=stats[:, h, :], in_=k_ps[:, h * DK:(h + 1) * DK])
                                nc.vector.bn_aggr(out=mv[:, h, :], in_=stats[:, h, :])
                                nc.vector.bn_stats(out=stats[:, 4 + h, :], in_=v_ps[:, h * DK:(h + 1) * DK])
                                nc.vector.bn_aggr(out=mv[:, 4 + h, :], in_=stats[:, 4 + h, :])
                            rstd = a_ln.tile([128, 8], f32, tag="rstd")
                            nc.scalar.activation(out=rstd[:], in_=mv[:, :, 1], func=ACT.Sqrt, bias=eps_t[:])
                            nc.vector.reciprocal(out=rstd[:], in_=rstd[:])

                            if not has_affine:
                                # delta/N folds into K's rstd
                                nc.vector.tensor_scalar_mul(
                                    out=rstd[:, 0:4], in0=rstd[:, 0:4],
                                    scalar1=delta_sb[:, gsub:gsub + 1])
                                nmr = a_ln.tile([128, 4], f32, tag="nmr")
                                nc.vector.tensor_mul(nmr[:], mv[:, 0:4, 0], rstd[:, 0:4])
                                nc.vector.tensor_scalar_mul(out=nmr[:], in0=nmr[:], scalar1=-1.0)
                                for h in range(4):
                                    ph = slice(h * DK, (h + 1) * DK)
                                    nc.scalar.activation(
                                        out=kf[:, ph], in_=k_ps[:, ph], func=ACT.Identity,
                                        bias=nmr[:, h:h + 1], scale=rstd[:, h:h + 1])
                                    nc.vector.tensor_scalar(
                                        out=vf[:, ph], in0=v_ps[:, ph],
                                        scalar1=mv[:, 4 + h, 0:1], scalar2=rstd[:, 4 + h:5 + h],
                                        op0=sub_, op1=mult)
                            else:
                                gb = slice(oc * 512, (oc + 1) * 512)
                                for h in range(4):
                                    ph = slice(h * DK, (h + 1) * DK)
                                    nc.vector.tensor_scalar(
                                        out=kf[:, ph], in0=k_ps[:, ph],
                                        scalar1=mv[:, h, 0:1], scalar2=rstd[:, h:h + 1],
                                        op0=sub_, op1=mult)
                                    nc.vector.tensor_scalar(
                                        out=vf[:, ph], in0=v_ps[:, ph],
                                        scalar1=mv[:, 4 + h, 0:1], scalar2=rstd[:, 4 + h:5 + h],
                                        op0=sub_, op1=mult)
                                nc.vector.tensor_mul(kf[:], kf[:], gamma_sb[:, gb])
                                nc.vector.tensor_add(kf[:], kf[:], beta_sb[:, gb])
                                nc.vector.tensor_scalar_mul(
                                    out=kf[:], in0=kf[:],
                                    scalar1=delta_sb[:, gsub:gsub + 1])
                                nc.vector.tensor_mul(vf[:], vf[:], gamma_sb[:, gb])
                                nc.vector.tensor_add(vf[:], vf[:], beta_sb[:, gb])

                            halves.append((kf, vf))

                        if pending is not None:
                            emit_scores(pending)
                        if s == 0 and sb == 0:
                            # Allocate this batch's scores PSUM only after the
                            # previous batch's deferred drain was emitted (the
                            # single-buffer pool makes the memset wait on it).
                            sp = a_sps.tile([128, H, DK], f32, tag="scores")
                            scores_tiles[b] = sp
                            nc.vector.memset(sp, 0.0)
                        pending = (halves, b,
                                   s == NSUP - 1 and sb == NSUB - 1)

            emit_scores(pending)

        # ------- Phase B (fused): qT, attn, x1T, FFN1 -> g, FFN2+bypass ---
        with (
            tc.tile_pool(name="b_sc", bufs=2) as b_sc,
            tc.tile_pool(name="b_x", bufs=2) as b_x,
            tc.tile_pool(name="b_xb", bufs=2) as b_xb,
            tc.tile_pool(name="b_q", bufs=1) as b_q,
            tc.tile_pool(name="b_x1", bufs=2) as b_x1,
            tc.tile_pool(name="b_x18", bufs=2) as b_x18,
            tc.tile_pool(name="b_g", bufs=1) as b_g,
            tc.tile_pool(name="b_s", bufs=4) as b_s,
            tc.tile_pool(name="b_y", bufs=2) as b_y,
            tc.tile_pool(name="b_o", bufs=1) as b_o,
            tc.tile_pool(name="b_qps", bufs=2, space="PSUM") as b_qps,
            tc.tile_pool(name="b_aps", bufs=2, space="PSUM") as b_aps,
            tc.tile_pool(name="b_hps", bufs=2, space="PSUM") as b_hps,
            tc.tile_pool(name="b_yps", bufs=2, space="PSUM") as b_yps,
        ):
            FCH = FC // 2
            for b in range(B):
                # Stage this batch's reduced scores just in time, on the
                # scalar DMA queue so the sync queue's x prefetches are
                # never stuck behind the AllReduce dependency.
                sc_f = b_sc.tile([128, H * DK], f32, tag="scf")
                nc.scalar.dma_start(out=sc_f[:], in_=cc_out[b])
                nc.vector.tensor_copy(out=scores_bf[:, b, :, :], in_=sc_f[:])

                for s in range(NSUP):
                    xtb8 = b_x.tile([128, KC, SUP], f8, tag="xtb1")
                    nc.sync.dma_start(out=xtb8[:], in_=xTb_d.ap()[b, s])
                    xtbb = b_xb.tile([128, KC, SUP], bf16, tag="xtbb")
                    nc.sync.dma_start(out=xtbb[:], in_=xTbBf_d.ap()[b, s])

                    qt = b_q.tile([128, H, SUP], bf16, tag="qt")
                    for m in range(KC):
                        q_ps = b_qps.tile([128, SUP], f32, tag="qps")
                        for kc in range(0, KC, 2):
                            nc.tensor.matmul(
                                q_ps[:], lhsT=wq_sb[:, kc:kc + 2, m * 128:(m + 1) * 128],
                                rhs=xtb8[:, kc:kc + 2, :],
                                start=(kc == 0), stop=(kc == KC - 2),
                                perf_mode=DR)
                        nc.scalar.activation(out=qt[:, m, :], in_=q_ps[:],
                                             func=ACT.Identity, bias=bq_sb[:, m:m + 1],
                                             scale=1.0 / SW)

                    x1b = b_x1.tile([128, KC, SUP], bf16, tag="x1")
                    x18 = b_x18.tile([128, KC, SUP], f8, tag="x18")
                    for h in range(H):
                        a_ps = b_aps.tile([128, SUP], f32, tag="aps")
                        nc.tensor.matmul(a_ps[:], lhsT=scores_bf[:, b, h, :],
                                         rhs=qt[:, h, :], start=True, stop=True)
                        nc.vector.tensor_add(x1b[:, h, :], a_ps[:], xtbb[:, h, :])
                        nc.scalar.activation(out=x18[:, h, :], in_=x1b[:, h, :],
                                             func=ACT.Copy)

                    g8 = b_g.tile([128, FC, SUP], f8, tag="g8")
                    for m in range(FC):
                        h_ps = b_hps.tile([128, SUP], f32, tag="hps")
                        for kc in range(0, KC, 2):
                            nc.tensor.matmul(
                                h_ps[:], lhsT=w1_sb[:, kc:kc + 2, m * 128:(m + 1) * 128],
                                rhs=x18[:, kc:kc + 2, :],
                                start=(kc == 0), stop=(kc == KC - 2),
                                perf_mode=DR)
                        st = b_s.tile([128, SUP], f32, tag="silu")
                        nc.scalar.activation(out=st[:], in_=h_ps[:], func=ACT.Silu,
                                             bias=b1_sb[:, m:m + 1], scale=1.0 / SW)
                        if has_b1:
                            # g = silu(z) - z/2 with z = ps/SW + b1
                            hz = b_s.tile([128, SUP], f32, tag="hz")
                            nc.scalar.activation(out=hz[:], in_=h_ps[:], func=ACT.Identity,
                                                 bias=b1h_sb[:, m:m + 1], scale=0.5 / SW)
                            nc.vector.scalar_tensor_tensor(
                                out=g8[:, m, :], in0=hz[:], scalar=-1.0,
                                in1=st[:], op0=mult, op1=add_)
                        else:
                            nc.vector.scalar_tensor_tensor(
                                out=g8[:, m, :], in0=h_ps[:], scalar=-0.5 / SW,
                                in1=st[:], op0=mult, op1=add_)

                    ot = b_o.tile([128, KC, SUP], bf16, tag="ot")
                    for m in range(KC):
                        y_ps = b_yps.tile([128, SUP], f32, tag="yps")
                        for kc in range(0, FC, 2):
                            nc.tensor.matmul(
                                y_ps[:], lhsT=w2_sb[:, kc:kc + 2, m * 128:(m + 1) * 128],
                                rhs=g8[:, kc:kc + 2, :],
                                start=(kc == 0), stop=False, perf_mode=DR)
                        # Precise bf16 bypass: + x1 @ (SW*0.5*W1@W2)
                        for kc in range(KC):
                            nc.tensor.matmul(
                                y_ps[:], lhsT=w12_sb[:, kc, m * 128:(m + 1) * 128],
                                rhs=x1b[:, kc, :],
                                start=False, stop=(kc == KC - 1))
                        yt = b_y.tile([128, SUP], f32, tag="yt")
                        if has_b2:
                            nc.scalar.activation(out=yt[:], in_=y_ps[:], func=ACT.Identity,
                                                 bias=b2_sb[:, m:m + 1], scale=1.0 / SW)
                        else:
                            nc.scalar.activation(out=yt[:], in_=y_ps[:], func=ACT.Copy,
                                                 scale=1.0 / SW)
                        nc.vector.tensor_add(ot[:, m, :], yt[:], x1b[:, m, :])
                    nc.gpsimd.dma_start(out=out_d.ap()[b, s], in_=ot[:])

        w_ab1_cm.__exit__(None, None, None)
        w_b2a_cm.__exit__(None, None, None)

    nc.finalize()
    return nc


def _get_graph(flags):
    if flags not in _GRAPH_CACHE:
        _GRAPH_CACHE[flags] = _build(flags)
    return _GRAPH_CACHE[flags]


def kernel(x, delta_x, Wq, bq, Wk, bk, Wv, bv, gamma_k, beta_k, W1, b1, W2, b2,
           _trace=False):
    from concourse.bass_utils import run_bass_kernel_spmd

    bf = ml_dtypes.bfloat16
    f8 = ml_dtypes.float8_e4m3
    x = np.asarray(x, np.float32)
    delta_x = np.asarray(delta_x, np.float32)
    Wq, Wk, Wv = (np.asarray(w, np.float32) for w in (Wq, Wk, Wv))
    W1, W2 = np.asarray(W1, np.float32), np.asarray(W2, np.float32)
    bq, bk, bv = (np.asarray(v, np.float32) for v in (bq, bk, bv))
    b1, b2 = np.asarray(b1, np.float32), np.asarray(b2, np.float32)
    gamma_k = np.asarray(gamma_k, np.float32)
    beta_k = np.asarray(beta_k, np.float32)

    has_bk = bool(np.any(bk))
    has_bv = bool(np.any(bv))
    has_b1 = bool(np.any(b1))
    has_b2 = bool(np.any(b2))
    has_affine = not (np.all(gamma_k == 1.0) and np.all(beta_k == 0.0))
    flags = (has_bk, has_bv, has_b1, has_b2, has_affine)
    nc = _get_graph(flags)

    sw = np.float32(SW)

    def wtile(W, nchunks):
        # [nchunks*128, F] -> SBUF layout [128, nchunks, F]
        return np.ascontiguousarray(W.reshape(nchunks, 128, -1).transpose(1, 0, 2))

    wq_8 = wtile((Wq * sw).astype(f8), KC)
    wk_8 = wtile((Wk * sw).astype(f8), KC)
    wv_8 = wtile((Wv * sw).astype(f8), KC)
    w1_8 = wtile((W1 * sw).astype(f8), KC)
    w2_8 = wtile((W2 * sw).astype(f8), FC)
    w12_b = wtile((np.float32(0.5) * sw * (W1 @ W2)).astype(bf), KC)
    bq_t = np.ascontiguousarray(bq.reshape(KC, 128).T)
    b1_t = np.ascontiguousarray(b1.reshape(FC, 128).T)
    delta_pre = (delta_x / np.float32(N)).astype(np.float32)

    in_maps = []
    for c in range(N_CORES):
        t0 = c * NT
        # [B, NT, D] -> [B, NSUP, 128, KC, SUP]: [b,s,p,kc,t] = x[b, s*SUP+t, kc*128+p]
        xT = np.ascontiguousarray(
            x[:, t0:t0 + NT, :].reshape(B, NSUP, SUP, KC, 128).transpose(0, 1, 4, 3, 2))
        m = {"xTb": xT.astype(f8), "xTbBf": xT.astype(bf),
             "delta": np.ascontiguousarray(
                 delta_pre[t0:t0 + NT].reshape(NT // 128, 128).T),
             "Wq": wq_8, "Wk": wk_8, "Wv": wv_8, "W1": w1_8, "W2": w2_8,
             "W12": w12_b, "bq": bq_t, "b1": b1_t}
        if has_b1:
            m["b1h"] = np.ascontiguousarray((b1 * np.float32(0.5)).reshape(FC, 128).T)
        if has_bk:
            m["bk"] = (bk * sw).astype(np.float32)
        if has_bv:
            m["bv"] = (bv * sw).astype(np.float32)
        if has_b2:
            m["b2"] = np.ascontiguousarray(b2.reshape(KC, 128).T)
        if has_affine:
            m["gamma"] = gamma_k.reshape(D).copy()
            m["beta"] = beta_k.reshape(D).copy()
        in_maps.append(m)

    res = run_bass_kernel_spmd(nc, in_maps, core_ids=list(range(N_CORES)),
                               trace=_trace)

    out = np.empty((B, N, D), np.float32)
    for c in range(N_CORES):
        t0 = c * NT
        # [B, NSUP, 128, KC, SUP] -> [B, NT, D]
        ot = res.results[c]["outT"].transpose(0, 1, 4, 3, 2).astype(np.float32)
        out[:, t0:t0 + NT, :] = ot.reshape(B, NT, D)
    if _trace:
        return out, res
    return out
